# revision 8
# baseline (speedup 1.0000x reference)
"""ChebConv (K=4) distributed Trainium2 kernel — 8 NeuronCores.

Strategy:
  - x (B,Cin,V) is reshaped host-side to a node-major feature table
    x0[(V, F=B*Cin)] (bf16) and replicated to all 8 cores as the gather
    table for spmm step 1.
  - Edges are sorted by destination row; rows are sharded across the 8
    cores (6272 rows each, V padded to 50176). Each core processes only
    the edges landing in its rows.
  - Each spmm step: dma_gather (MoE gather instruction, 1KB descriptors)
    pulls x[col] rows for chunks of 128 edges; a per-chunk segment
    matrix M[e, r] = val[e] * (row[e] == r) is built on the vector
    engine; PE matmul M^T @ z accumulates each 128-row tile in PSUM.
    The Chebyshev recurrence (x2 = 2*A@x1 - x0) folds the factor 2 into
    the edge values and becomes a single subtract in the epilogue.
  - After steps 1 and 2 an 8-core AllGather rebuilds the full node table
    (the "halo exchange") used as the next step's gather source.
  - The K-contraction einsum runs per row-tile right after step 3:
    PE-transpose of each (v,c) tile then matmul against the replicated
    weights, bias added on the scalar engine, f32 result DMA'd out.

int16 gather indices limit a table to 32768 rows, so the node table is
split in two halves (25088 rows each); edges are grouped host-side by
(row-tile, col-half) and padded to multiples of 128.
"""

import os
import numpy as np
import ml_dtypes

import concourse.bacc as bacc
import concourse.bass as bass
import concourse.mybir as mybir
import concourse.tile as tile
from concourse.bass_utils import run_bass_kernel_spmd
from concourse.masks import make_identity

# ----- problem constants (hardcoded per spec) -----
V = 50000
B = 4
CIN = 128
COUT = 128
K = 4
E = 800000
F = B * CIN  # 512
NCORES = 8
TILES_PER_CORE = 49
R = TILES_PER_CORE * 128          # 6272 rows per core
V_PAD = NCORES * R                # 50176
HALF = V_PAD // 2                 # 25088 (= 4 cores' rows)

MAX_BLK = int(os.environ.get("CHEB_MAXBLK", "8"))  # chunks per gather block
DEBUG_NOCC = os.environ.get("CHEB_NOCC", "") == "1"    # skip collectives
DEBUG_STEPS = int(os.environ.get("CHEB_STEPS", "3"))   # spmm steps to run

USE_BF16 = os.environ.get("CHEB_F32", "") != "1"
DT = mybir.dt.bfloat16 if USE_BF16 else mybir.dt.float32
NPDT = ml_dtypes.bfloat16 if USE_BF16 else np.float32

LAST_RESULT = None  # test harness reads exec_time_ns from here


# ---------------------------------------------------------------------------
# host-side edge preprocessing
# ---------------------------------------------------------------------------

class _Block:
    __slots__ = ("half", "icol", "cg0", "n", "chunks")

    def __init__(self, half, icol, cg0, n, chunks):
        self.half = half      # 0 = cols [0, HALF), 1 = cols [HALF, V_PAD)
        self.icol = icol      # column offset into the idx sbuf tensor
        self.cg0 = cg0        # first global chunk id of this block
        self.n = n            # number of 128-edge chunks
        self.chunks = chunks  # list of (tile, is_first_of_tile, is_last_of_tile)


def _preprocess(edge_row, edge_col, edge_vals):
    """Group/pad edges per (core, row-tile, col-half).

    Chunk counts are equalized across cores (max) so all cores run the
    same instruction graph; short cores get zero-valued padding edges.
    Returns (blocks, NCH, IDXCOLS, per-core packed arrays).
    """
    core = edge_row // R
    tile_id = (edge_row % R) // 128
    half = (edge_col >= HALF).astype(np.int64)

    # bucket edges by (core, tile, half)
    key = (core * TILES_PER_CORE + tile_id) * 2 + half
    order = np.argsort(key, kind="stable")
    skey = key[order]
    srow = edge_row[order]
    scol = edge_col[order]
    sval = edge_vals[order]
    nkeys = NCORES * TILES_PER_CORE * 2
    counts = np.bincount(skey, minlength=nkeys).reshape(NCORES, TILES_PER_CORE, 2)
    starts = np.zeros(nkeys + 1, dtype=np.int64)
    np.cumsum(counts.reshape(-1), out=starts[1:])

    # chunks per (tile, half): max over cores, >=1 for half 0
    nchunk = np.ceil(counts / 128.0).astype(np.int64).max(axis=0)  # (T, 2)
    nchunk[:, 0] = np.maximum(nchunk[:, 0], 1)

    # static block/chunk map (identical for every core)
    blocks = []
    cg = 0
    icol = 0
    chunk_tile = []   # per global chunk: tile id
    for t in range(TILES_PER_CORE):
        for h in range(2):
            n = int(nchunk[t, h])
            j = 0
            while j < n:
                bn = min(MAX_BLK, n - j)
                blocks.append(_Block(h, icol, cg, bn, []))
                for _ in range(bn):
                    chunk_tile.append(t)
                    blocks[-1].chunks.append(t)
                    cg += 1
                    j += 1
                icol += bn * 8
    NCH = cg
    IDXCOLS = icol

    # mark first/last chunk per tile
    first_of_tile = {}
    last_of_tile = {}
    for i, t in enumerate(chunk_tile):
        if t not in first_of_tile:
            first_of_tile[t] = i
        last_of_tile[t] = i
    for blk in blocks:
        marked = []
        base = blk.cg0
        for j, t in enumerate(blk.chunks):
            i = base + j
            marked.append((t, i == first_of_tile[t], i == last_of_tile[t]))
        blk.chunks = marked

    # per-core packed arrays
    per_core = []
    for c in range(NCORES):
        idx_np = np.zeros((128, IDXCOLS), dtype=np.int16)
        val_np = np.zeros((128, NCH), dtype=np.float32)
        roff_np = np.zeros((128, NCH), dtype=np.float32)
        for t in range(TILES_PER_CORE):
            base_row = c * R + t * 128
            for h in range(2):
                kidx = (c * TILES_PER_CORE + t) * 2 + h
                s, e = starts[kidx], starts[kidx + 1]
                cnt = e - s
                col = scol[s:e].astype(np.int64) - h * HALF
                row = srow[s:e].astype(np.int64) - base_row
                val = sval[s:e]
                n = int(nchunk[t, h])
                pad = n * 128 - cnt
                if pad:
                    col = np.concatenate([col, np.zeros(pad, dtype=np.int64)])
                    row = np.concatenate([row, np.zeros(pad, dtype=np.int64)])
                    val = np.concatenate([val, np.zeros(pad, dtype=np.float32)])
                # locate this (t,h)'s chunks in the global order: they are
                # consecutive; find their global ids / icols via the blocks
                # we built in the same traversal order.
                per_core_write(idx_np, val_np, roff_np, t, h, col, row, val,
                               blocks, nchunk)
        per_core.append((idx_np, val_np, roff_np))
    return blocks, NCH, IDXCOLS, nchunk, per_core


def per_core_write(idx_np, val_np, roff_np, t, h, col, row, val, blocks, nchunk):
    # global chunk id of (t, h)'s first chunk = sum of all earlier (t', h')
    cg0 = 0
    for tt in range(t):
        cg0 += int(nchunk[tt, 0]) + int(nchunk[tt, 1])
    if h == 1:
        cg0 += int(nchunk[t, 0])
    n = int(nchunk[t, h])
    # icol of that chunk: consistent traversal => recompute identically
    icol0 = cg0 * 8
    ncols = n * 8
    # idx layout: edge i of the (t,h) group -> within-block local position.
    # Blocks of this group are consecutive chunks; since icol advances by
    # 8 per chunk globally, local i maps to col icol0 + i//16, partition
    # (i%16) + 16*g for all 8 gpsimd groups g.
    i = np.arange(n * 128)
    cols = icol0 + i // 16
    parts = i % 16
    for g in range(8):
        idx_np[parts + 16 * g, cols] = col.astype(np.int16)
    # val/rowoff layout: (partition=edge%128, col=global chunk id)
    ch = cg0 + i // 128
    p = i % 128
    val_np[p, ch] = val
    roff_np[p, ch] = row.astype(np.float32)


# ---------------------------------------------------------------------------
# device graph
# ---------------------------------------------------------------------------

def _build_nc(blocks, NCH, IDXCOLS):
    nc = bacc.Bacc("TRN2", target_bir_lowering=False, debug=False,
                   num_devices=NCORES)
    f32 = mybir.dt.float32

    # ---- I/O ----
    x0_tab = nc.dram_tensor("x0_tab", [V_PAD, F], DT, kind="ExternalInput")
    x0_own = nc.dram_tensor("x0_own", [R, F], DT, kind="ExternalInput")
    idxs_d = nc.dram_tensor("idxs", [128, IDXCOLS], mybir.dt.int16,
                            kind="ExternalInput")
    val1_d = nc.dram_tensor("val1", [128, NCH], DT, kind="ExternalInput")
    val2_d = nc.dram_tensor("val2", [128, NCH], DT, kind="ExternalInput")
    roff_d = nc.dram_tensor("roff", [128, NCH], DT, kind="ExternalInput")
    iota_d = nc.dram_tensor("iota", [128, 128], DT, kind="ExternalInput")
    w_d = nc.dram_tensor("w", [CIN, K * COUT], DT, kind="ExternalInput")
    bias_d = nc.dram_tensor("bias", [COUT, 1], f32, kind="ExternalInput")
    out_d = nc.dram_tensor("out", [B, COUT, R], f32, kind="ExternalOutput")

    # ---- internal DRAM ----
    x1_own_d = nc.dram_tensor("x1_own_d", [R, F], DT)
    x2_own_d = nc.dram_tensor("x2_own_d", [R, F], DT)
    x1_tab = nc.dram_tensor("x1_tab", [V_PAD, F], DT, addr_space="Shared")
    x2_tab = nc.dram_tensor("x2_tab", [V_PAD, F], DT, addr_space="Shared")

    rg = [list(range(NCORES))]

    with tile.TileContext(nc) as tc:
        with (
            tc.tile_pool(name="const", bufs=1) as constp,
            tc.tile_pool(name="zp", bufs=4) as zp,
            tc.tile_pool(name="mp", bufs=4) as mp,
            tc.tile_pool(name="xown", bufs=3) as xownp,
            tc.tile_pool(name="xstr", bufs=4) as xstrp,
            tc.tile_pool(name="x3p", bufs=3) as x3p,
            tc.tile_pool(name="xkT", bufs=3) as xkTp,
            tc.tile_pool(name="outp", bufs=3) as outp,
            tc.tile_pool(name="ps_seg", bufs=3, space="PSUM") as ps_seg,
            tc.tile_pool(name="ps_tp", bufs=2, space="PSUM") as ps_tp,
            tc.tile_pool(name="ps_o", bufs=2, space="PSUM") as ps_o,
        ):
            # ---- preload constants ----
            idxs_sb = constp.tile([128, IDXCOLS], mybir.dt.int16)
            nc.sync.dma_start(idxs_sb[:], idxs_d[:])
            val1_sb = constp.tile([128, NCH], DT)
            nc.sync.dma_start(val1_sb[:], val1_d[:])
            val2_sb = constp.tile([128, NCH], DT)
            nc.sync.dma_start(val2_sb[:], val2_d[:])
            roff_sb = constp.tile([128, NCH], DT)
            nc.sync.dma_start(roff_sb[:], roff_d[:])
            iota_sb = constp.tile([128, 128], DT)
            nc.sync.dma_start(iota_sb[:], iota_d[:])
            w_sb = constp.tile([CIN, K * COUT], DT)
            nc.sync.dma_start(w_sb[:], w_d[:])
            bias_sb = constp.tile([COUT, 1], f32)
            nc.sync.dma_start(bias_sb[:], bias_d[:])
            ident_sb = constp.tile([128, 128], DT)
            make_identity(nc, ident_sb[:])

            def spmm_step(step):
                """One A-application; returns nothing (epilogues inline)."""
                if step == 1:
                    tab = x0_tab
                    val_sb = val1_sb
                elif step == 2:
                    tab = x0_tab if DEBUG_NOCC else x1_tab
                    val_sb = val2_sb
                else:
                    tab = x0_tab if DEBUG_NOCC else x2_tab
                    val_sb = val2_sb
                tabs = (tab[0:HALF, :], tab[HALF:V_PAD, :])

                psums = {}
                for blk in blocks:
                    n = blk.n
                    z = zp.tile([128, MAX_BLK, F], DT, tag="z")
                    nidx = n * 128
                    nc.gpsimd.dma_gather(
                        z[:, 0:n, :],
                        tabs[blk.half][:],
                        idxs_sb[:, blk.icol:blk.icol + n * 8],
                        nidx,
                        nidx,
                        F,
                    )
                    m = mp.tile([128, MAX_BLK, 128], DT, tag="m")
                    nc.vector.tensor_tensor(
                        out=m[:, 0:n, :],
                        in0=roff_sb[:, blk.cg0:blk.cg0 + n, None].to_broadcast(
                            [128, n, 128]),
                        in1=iota_sb[:, None, :].to_broadcast([128, n, 128]),
                        op=mybir.AluOpType.is_equal,
                    )
                    nc.vector.tensor_tensor(
                        out=m[:, 0:n, :],
                        in0=m[:, 0:n, :],
                        in1=val_sb[:, blk.cg0:blk.cg0 + n, None].to_broadcast(
                            [128, n, 128]),
                        op=mybir.AluOpType.mult,
                    )
                    for j, (t, first, last) in enumerate(blk.chunks):
                        if first:
                            psums[t] = ps_seg.tile([128, F], f32, tag="seg",
                                                   name="seg")
                        nc.tensor.matmul(
                            psums[t][:],
                            lhsT=m[:, j, :],
                            rhs=z[:, j, :],
                            start=first,
                            stop=last,
                        )
                        if last:
                            epilogue(step, t, psums.pop(t))

            def epilogue(step, t, psum):
                sl = slice(t * 128, (t + 1) * 128)
                if step == 1:
                    xo = xownp.tile([128, F], DT, tag="xo")
                    nc.vector.tensor_copy(out=xo[:], in_=psum[:])
                    nc.sync.dma_start(x1_own_d[sl, :], xo[:])
                elif step == 2:
                    x0t = xstrp.tile([128, F], DT, tag="xs")
                    nc.sync.dma_start(x0t[:], x0_own[sl, :])
                    xo = xownp.tile([128, F], DT, tag="xo")
                    nc.vector.tensor_sub(out=xo[:], in0=psum[:], in1=x0t[:])
                    nc.sync.dma_start(x2_own_d[sl, :], xo[:])
                else:
                    x1t = xstrp.tile([128, F], DT, tag="xs")
                    nc.sync.dma_start(x1t[:], x1_own_d[sl, :])
                    x3t = x3p.tile([128, F], DT, tag="x3")
                    nc.vector.tensor_sub(out=x3t[:], in0=psum[:], in1=x1t[:])
                    einsum_tile(t, x3t)

            def einsum_tile(t, x3t):
                sl = slice(t * 128, (t + 1) * 128)
                x0t = xstrp.tile([128, F], DT, tag="xs")
                nc.sync.dma_start(x0t[:], x0_own[sl, :])
                x1t = xstrp.tile([128, F], DT, tag="xs")
                nc.sync.dma_start(x1t[:], x1_own_d[sl, :])
                x2t = xstrp.tile([128, F], DT, tag="xs")
                nc.sync.dma_start(x2t[:], x2_own_d[sl, :])
                srcs = (x0t, x1t, x2t, x3t)
                for b in range(B):
                    po = ps_o.tile([128, 128], mybir.dt.float32, tag="po")
                    for k in range(K):
                        tp = ps_tp.tile([128, 128], DT, tag="tp")
                        nc.tensor.transpose(
                            tp[:], srcs[k][:, b * 128:(b + 1) * 128],
                            ident_sb[:])
                        xkT = xkTp.tile([128, 128], DT, tag="xkT")
                        nc.vector.tensor_copy(out=xkT[:], in_=tp[:])
                        nc.tensor.matmul(
                            po[:],
                            lhsT=w_sb[:, k * COUT:(k + 1) * COUT],
                            rhs=xkT[:],
                            start=(k == 0),
                            stop=(k == K - 1),
                        )
                    ob = outp.tile([128, 128], mybir.dt.float32, tag="ob")
                    nc.scalar.activation(
                        ob[:], po[:], mybir.ActivationFunctionType.Identity,
                        bias=bias_sb[:, 0:1])
                    nc.sync.dma_start(out_d[b, :, t * 128:(t + 1) * 128], ob[:])

            spmm_step(1)
            if not DEBUG_NOCC:
                nc.gpsimd.collective_compute(
                    "AllGather", mybir.AluOpType.bypass, replica_groups=rg,
                    ins=[x1_own_d.ap().opt()], outs=[x1_tab.ap().opt()])
            if DEBUG_STEPS >= 2:
                spmm_step(2)
                if not DEBUG_NOCC:
                    nc.gpsimd.collective_compute(
                        "AllGather", mybir.AluOpType.bypass, replica_groups=rg,
                        ins=[x2_own_d.ap().opt()], outs=[x2_tab.ap().opt()])
            if DEBUG_STEPS >= 3:
                spmm_step(3)

    nc.compile()
    return nc


# ---------------------------------------------------------------------------
# entry point
# ---------------------------------------------------------------------------

def kernel(x, edge_row, edge_col, edge_vals, weights, biases):
    global LAST_RESULT
    x = np.asarray(x, dtype=np.float32)
    edge_row = np.asarray(edge_row, dtype=np.int32)
    edge_col = np.asarray(edge_col, dtype=np.int32)
    edge_vals = np.asarray(edge_vals, dtype=np.float32)
    weights = np.asarray(weights, dtype=np.float32)
    biases = np.asarray(biases, dtype=np.float32)

    blocks, NCH, IDXCOLS, nchunk, per_core = _preprocess(
        edge_row.astype(np.int64), edge_col.astype(np.int64), edge_vals)

    # node-major feature table (V_PAD, F), b-major features
    x0 = np.transpose(x, (2, 0, 1)).reshape(V, F)
    x0p = np.zeros((V_PAD, F), dtype=np.float32)
    x0p[:V] = x0
    x0p = x0p.astype(NPDT)

    w_host = np.transpose(weights, (1, 0, 2)).reshape(CIN, K * COUT).astype(NPDT)
    bias_host = biases.reshape(COUT, 1).astype(np.float32)
    iota_host = np.broadcast_to(
        np.arange(128, dtype=np.float32)[None, :], (128, 128)).astype(NPDT).copy()

    nc = _build_nc(blocks, NCH, IDXCOLS)

    in_maps = []
    for c in range(NCORES):
        idx_np, val_np, roff_np = per_core[c]
        in_maps.append({
            "x0_tab": x0p,
            "x0_own": x0p[c * R:(c + 1) * R].copy(),
            "idxs": idx_np,
            "val1": val_np.astype(NPDT),
            "val2": (2.0 * val_np).astype(NPDT),
            "roff": roff_np.astype(NPDT),
            "iota": iota_host,
            "w": w_host,
            "bias": bias_host,
        })

    res = run_bass_kernel_spmd(nc, in_maps, list(range(NCORES)))
    LAST_RESULT = res
    out = np.concatenate([res.results[c]["out"] for c in range(NCORES)], axis=2)
    return np.ascontiguousarray(out[:, :, :V]).astype(np.float32)


# revision 15
# speedup vs baseline: 1.0547x; 1.0547x over previous
"""ChebConv (K=4) distributed Trainium2 kernel — 8 NeuronCores.

Strategy:
  - x (B,Cin,V) is reshaped host-side to a node-major feature table
    x0[(V, F=B*Cin)] (bf16) and replicated to all 8 cores as the gather
    table for spmm step 1.
  - Edges are sorted by destination row; rows are sharded across the 8
    cores (6272 rows each, V padded to 50176). Each core processes only
    the edges landing in its rows.
  - Each spmm step: dma_gather (MoE gather instruction, 1KB descriptors)
    pulls x[col] rows for chunks of 128 edges; a per-chunk segment
    matrix M[e, r] = val[e] * (row[e] == r) is built on the vector
    engine; PE matmul M^T @ z accumulates each 128-row tile in PSUM.
    The Chebyshev recurrence (x2 = 2*A@x1 - x0) folds the factor 2 into
    the edge values and becomes a single subtract in the epilogue.
  - After steps 1 and 2 an 8-core AllGather rebuilds the full node table
    (the "halo exchange") used as the next step's gather source.
  - The K-contraction einsum runs per row-tile right after step 3:
    PE-transpose of each (v,c) tile then matmul against the replicated
    weights, bias added on the scalar engine, f32 result DMA'd out.

int16 gather indices limit a table to 32768 rows, so the node table is
split in two halves (25088 rows each); edges are grouped host-side by
(row-tile, col-half) and padded to multiples of 128.
"""

import os
import numpy as np
import ml_dtypes

import concourse.bacc as bacc
import concourse.bass as bass
import concourse.mybir as mybir
import concourse.tile as tile
from concourse.bass_utils import run_bass_kernel_spmd

# ----- problem constants (hardcoded per spec) -----
V = 50000
B = 4
CIN = 128
COUT = 128
K = 4
E = 800000
F = B * CIN  # 512
NCORES = 8
TILES_PER_CORE = 49
R = TILES_PER_CORE * 128          # 6272 rows per core
V_PAD = NCORES * R                # 50176
HALF = V_PAD // 2                 # 25088 (= 4 cores' rows)

MAX_BLK = int(os.environ.get("CHEB_MAXBLK", "8"))  # chunks per gather block
DEBUG_NOCC = os.environ.get("CHEB_NOCC", "") == "1"    # skip collectives
DEBUG_STEPS = int(os.environ.get("CHEB_STEPS", "3"))   # spmm steps to run

USE_BF16 = os.environ.get("CHEB_F32", "") != "1"
DT = mybir.dt.bfloat16 if USE_BF16 else mybir.dt.float32
NPDT = ml_dtypes.bfloat16 if USE_BF16 else np.float32

LAST_RESULT = None  # test harness reads exec_time_ns from here


# ---------------------------------------------------------------------------
# host-side edge preprocessing
# ---------------------------------------------------------------------------

class _Block:
    __slots__ = ("half", "icol", "cg0", "n", "chunks")

    def __init__(self, half, icol, cg0, n, chunks):
        self.half = half      # 0 = cols [0, HALF), 1 = cols [HALF, V_PAD)
        self.icol = icol      # column offset into the idx sbuf tensor
        self.cg0 = cg0        # first global chunk id of this block
        self.n = n            # number of 128-edge chunks
        self.chunks = chunks  # list of (tile, is_first_of_tile, is_last_of_tile)


def _preprocess(edge_row, edge_col, edge_vals):
    """Group/pad edges per (core, row-tile, col-half).

    Chunk counts are equalized across cores (max) so all cores run the
    same instruction graph; short cores get zero-valued padding edges.
    Returns (blocks, NCH, IDXCOLS, per-core packed arrays).
    """
    core = edge_row // R
    tile_id = (edge_row % R) // 128
    half = (edge_col >= HALF).astype(np.int64)

    # bucket edges by (core, tile, half)
    key = (core * TILES_PER_CORE + tile_id) * 2 + half
    order = np.argsort(key, kind="stable")
    skey = key[order]
    srow = edge_row[order]
    scol = edge_col[order]
    sval = edge_vals[order]
    nkeys = NCORES * TILES_PER_CORE * 2
    counts = np.bincount(skey, minlength=nkeys).reshape(NCORES, TILES_PER_CORE, 2)
    starts = np.zeros(nkeys + 1, dtype=np.int64)
    np.cumsum(counts.reshape(-1), out=starts[1:])

    # chunks per (tile, half): max over cores, >=1 for half 0
    nchunk = np.ceil(counts / 128.0).astype(np.int64).max(axis=0)  # (T, 2)
    nchunk[:, 0] = np.maximum(nchunk[:, 0], 1)

    # static block/chunk map (identical for every core)
    blocks = []
    cg = 0
    icol = 0
    chunk_tile = []   # per global chunk: tile id
    for t in range(TILES_PER_CORE):
        for h in range(2):
            n = int(nchunk[t, h])
            j = 0
            while j < n:
                bn = min(MAX_BLK, n - j)
                blocks.append(_Block(h, icol, cg, bn, []))
                for _ in range(bn):
                    chunk_tile.append(t)
                    blocks[-1].chunks.append(t)
                    cg += 1
                    j += 1
                icol += bn * 8
    NCH = cg
    IDXCOLS = icol

    # mark first/last chunk per tile
    first_of_tile = {}
    last_of_tile = {}
    for i, t in enumerate(chunk_tile):
        if t not in first_of_tile:
            first_of_tile[t] = i
        last_of_tile[t] = i
    for blk in blocks:
        marked = []
        base = blk.cg0
        for j, t in enumerate(blk.chunks):
            i = base + j
            marked.append((t, i == first_of_tile[t], i == last_of_tile[t]))
        blk.chunks = marked

    # per-core packed arrays
    per_core = []
    for c in range(NCORES):
        idx_np = np.zeros((128, IDXCOLS), dtype=np.int16)
        val_np = np.zeros((128, NCH), dtype=np.float32)
        roff_np = np.zeros((128, NCH), dtype=np.float32)
        for t in range(TILES_PER_CORE):
            base_row = c * R + t * 128
            for h in range(2):
                kidx = (c * TILES_PER_CORE + t) * 2 + h
                s, e = starts[kidx], starts[kidx + 1]
                cnt = e - s
                col = scol[s:e].astype(np.int64) - h * HALF
                row = srow[s:e].astype(np.int64) - base_row
                val = sval[s:e]
                n = int(nchunk[t, h])
                pad = n * 128 - cnt
                if pad:
                    col = np.concatenate([col, np.zeros(pad, dtype=np.int64)])
                    row = np.concatenate([row, np.zeros(pad, dtype=np.int64)])
                    val = np.concatenate([val, np.zeros(pad, dtype=np.float32)])
                # locate this (t,h)'s chunks in the global order: they are
                # consecutive; find their global ids / icols via the blocks
                # we built in the same traversal order.
                per_core_write(idx_np, val_np, roff_np, t, h, col, row, val,
                               blocks, nchunk)
        per_core.append((idx_np, val_np, roff_np))
    return blocks, NCH, IDXCOLS, nchunk, per_core


def per_core_write(idx_np, val_np, roff_np, t, h, col, row, val, blocks, nchunk):
    # global chunk id of (t, h)'s first chunk = sum of all earlier (t', h')
    cg0 = 0
    for tt in range(t):
        cg0 += int(nchunk[tt, 0]) + int(nchunk[tt, 1])
    if h == 1:
        cg0 += int(nchunk[t, 0])
    n = int(nchunk[t, h])
    # icol of that chunk: consistent traversal => recompute identically
    icol0 = cg0 * 8
    ncols = n * 8
    # idx layout: edge i of the (t,h) group -> within-block local position.
    # Blocks of this group are consecutive chunks; since icol advances by
    # 8 per chunk globally, local i maps to col icol0 + i//16, partition
    # (i%16) + 16*g for all 8 gpsimd groups g.
    i = np.arange(n * 128)
    cols = icol0 + i // 16
    parts = i % 16
    for g in range(8):
        idx_np[parts + 16 * g, cols] = col.astype(np.int16)
    # val/rowoff layout: (partition=edge%128, col=global chunk id)
    ch = cg0 + i // 128
    p = i % 128
    val_np[p, ch] = val
    roff_np[p, ch] = row.astype(np.float32)


# ---------------------------------------------------------------------------
# device graph
# ---------------------------------------------------------------------------

def _build_nc(blocks, NCH, IDXCOLS):
    nc = bacc.Bacc("TRN2", target_bir_lowering=False, debug=False,
                   num_devices=NCORES, num_swdge_queues=4)
    f32 = mybir.dt.float32

    # ---- I/O ----
    x0_tab = nc.dram_tensor("x0_tab", [V_PAD, F], DT, kind="ExternalInput")
    x0_own = nc.dram_tensor("x0_own", [R, F], DT, kind="ExternalInput")
    idxs_d = nc.dram_tensor("idxs", [128, IDXCOLS], mybir.dt.int16,
                            kind="ExternalInput")
    val1_d = nc.dram_tensor("val1", [128, NCH], DT, kind="ExternalInput")
    val2_d = nc.dram_tensor("val2", [128, NCH], DT, kind="ExternalInput")
    roff_d = nc.dram_tensor("roff", [128, NCH], DT, kind="ExternalInput")
    iota_d = nc.dram_tensor("iota", [128, 128], DT, kind="ExternalInput")
    ident_d = nc.dram_tensor("ident", [128, 128], DT, kind="ExternalInput")
    w_d = nc.dram_tensor("w", [CIN, K * COUT], DT, kind="ExternalInput")
    bias_d = nc.dram_tensor("bias", [COUT, 1], f32, kind="ExternalInput")
    out_d = nc.dram_tensor("out", [B, COUT, R], f32, kind="ExternalOutput")

    # ---- internal DRAM ----
    x1_own_d = nc.dram_tensor("x1_own_d", [R, F], DT)
    x2_own_d = nc.dram_tensor("x2_own_d", [R, F], DT)
    x1_tab = nc.dram_tensor("x1_tab", [V_PAD, F], DT, addr_space="Shared")
    x2_tab = nc.dram_tensor("x2_tab", [V_PAD, F], DT, addr_space="Shared")

    rg = [list(range(NCORES))]

    with tile.TileContext(nc) as tc:
        with (
            tc.tile_pool(name="const", bufs=1) as constp,
            tc.tile_pool(name="zp", bufs=4) as zp,
            tc.tile_pool(name="mp", bufs=4) as mp,
            tc.tile_pool(name="xown", bufs=3) as xownp,
            tc.tile_pool(name="xstr", bufs=4) as xstrp,
            tc.tile_pool(name="x3p", bufs=3) as x3p,
            tc.tile_pool(name="xkT", bufs=3) as xkTp,
            tc.tile_pool(name="outp", bufs=3) as outp,
            tc.tile_pool(name="ps_seg", bufs=3, space="PSUM") as ps_seg,
            tc.tile_pool(name="ps_tp", bufs=2, space="PSUM") as ps_tp,
            tc.tile_pool(name="ps_o", bufs=2, space="PSUM") as ps_o,
        ):
            # ---- preload constants ----
            idxs_sb = constp.tile([128, IDXCOLS], mybir.dt.int16)
            nc.sync.dma_start(idxs_sb[:], idxs_d[:])
            val1_sb = constp.tile([128, NCH], DT)
            nc.sync.dma_start(val1_sb[:], val1_d[:])
            val2_sb = constp.tile([128, NCH], DT)
            nc.sync.dma_start(val2_sb[:], val2_d[:])
            roff_sb = constp.tile([128, NCH], DT)
            nc.sync.dma_start(roff_sb[:], roff_d[:])
            iota_sb = constp.tile([128, 128], DT)
            nc.sync.dma_start(iota_sb[:], iota_d[:])
            w_sb = constp.tile([CIN, K * COUT], DT)
            nc.sync.dma_start(w_sb[:], w_d[:])
            bias_sb = constp.tile([COUT, 1], f32)
            nc.sync.dma_start(bias_sb[:], bias_d[:])
            ident_sb = constp.tile([128, 128], DT)
            nc.sync.dma_start(ident_sb[:], ident_d[:])

            def spmm_step(step):
                """One A-application; returns nothing (epilogues inline)."""
                if step == 1:
                    tab = x0_tab
                    val_sb = val1_sb
                elif step == 2:
                    tab = x0_tab if DEBUG_NOCC else x1_tab
                    val_sb = val2_sb
                else:
                    tab = x0_tab if DEBUG_NOCC else x2_tab
                    val_sb = val2_sb
                tabs = (tab[0:HALF, :], tab[HALF:V_PAD, :])

                psums = {}
                for bi, blk in enumerate(blocks):
                    n = blk.n
                    z = zp.tile([128, MAX_BLK, F], DT, tag="z")
                    nidx = n * 128
                    nc.gpsimd.dma_gather(
                        z[:, 0:n, :],
                        tabs[blk.half][:],
                        idxs_sb[:, blk.icol:blk.icol + n * 8],
                        nidx,
                        nidx,
                        F,
                        queue_num=bi % 4,
                    )
                    m = mp.tile([128, MAX_BLK, 128], DT, tag="m")
                    nc.vector.tensor_tensor(
                        out=m[:, 0:n, :],
                        in0=roff_sb[:, blk.cg0:blk.cg0 + n, None].to_broadcast(
                            [128, n, 128]),
                        in1=iota_sb[:, None, :].to_broadcast([128, n, 128]),
                        op=mybir.AluOpType.is_equal,
                    )
                    nc.vector.tensor_tensor(
                        out=m[:, 0:n, :],
                        in0=m[:, 0:n, :],
                        in1=val_sb[:, blk.cg0:blk.cg0 + n, None].to_broadcast(
                            [128, n, 128]),
                        op=mybir.AluOpType.mult,
                    )
                    for j, (t, first, last) in enumerate(blk.chunks):
                        if first:
                            psums[t] = ps_seg.tile([128, F], f32, tag="seg",
                                                   name="seg")
                        nc.tensor.matmul(
                            psums[t][:],
                            lhsT=m[:, j, :],
                            rhs=z[:, j, :],
                            start=first,
                            stop=last,
                        )
                        if last:
                            epilogue(step, t, psums.pop(t))

            def epilogue(step, t, psum):
                sl = slice(t * 128, (t + 1) * 128)
                if step == 1:
                    xo = xownp.tile([128, F], DT, tag="xo")
                    nc.vector.tensor_copy(out=xo[:], in_=psum[:])
                    nc.sync.dma_start(x1_own_d[sl, :], xo[:])
                elif step == 2:
                    x0t = xstrp.tile([128, F], DT, tag="xs")
                    nc.sync.dma_start(x0t[:], x0_own[sl, :])
                    xo = xownp.tile([128, F], DT, tag="xo")
                    nc.vector.tensor_sub(out=xo[:], in0=psum[:], in1=x0t[:])
                    nc.sync.dma_start(x2_own_d[sl, :], xo[:])
                else:
                    x1t = xstrp.tile([128, F], DT, tag="xs")
                    nc.sync.dma_start(x1t[:], x1_own_d[sl, :])
                    x3t = x3p.tile([128, F], DT, tag="x3")
                    nc.vector.tensor_sub(out=x3t[:], in0=psum[:], in1=x1t[:])
                    einsum_tile(t, x3t)

            def einsum_tile(t, x3t):
                sl = slice(t * 128, (t + 1) * 128)
                x0t = xstrp.tile([128, F], DT, tag="xs")
                nc.sync.dma_start(x0t[:], x0_own[sl, :])
                x1t = xstrp.tile([128, F], DT, tag="xs")
                nc.sync.dma_start(x1t[:], x1_own_d[sl, :])
                x2t = xstrp.tile([128, F], DT, tag="xs")
                nc.sync.dma_start(x2t[:], x2_own_d[sl, :])
                srcs = (x0t, x1t, x2t, x3t)
                for b in range(B):
                    po = ps_o.tile([128, 128], mybir.dt.float32, tag="po")
                    for k in range(K):
                        tp = ps_tp.tile([128, 128], DT, tag="tp")
                        nc.tensor.transpose(
                            tp[:], srcs[k][:, b * 128:(b + 1) * 128],
                            ident_sb[:])
                        xkT = xkTp.tile([128, 128], DT, tag="xkT")
                        nc.vector.tensor_copy(out=xkT[:], in_=tp[:])
                        nc.tensor.matmul(
                            po[:],
                            lhsT=w_sb[:, k * COUT:(k + 1) * COUT],
                            rhs=xkT[:],
                            start=(k == 0),
                            stop=(k == K - 1),
                        )
                    ob = outp.tile([128, 128], mybir.dt.float32, tag="ob")
                    nc.scalar.activation(
                        ob[:], po[:], mybir.ActivationFunctionType.Identity,
                        bias=bias_sb[:, 0:1])
                    nc.sync.dma_start(out_d[b, :, t * 128:(t + 1) * 128], ob[:])

            spmm_step(1)
            if not DEBUG_NOCC:
                nc.gpsimd.collective_compute(
                    "AllGather", mybir.AluOpType.bypass, replica_groups=rg,
                    ins=[x1_own_d.ap().opt()], outs=[x1_tab.ap().opt()])
            if DEBUG_STEPS >= 2:
                spmm_step(2)
                if not DEBUG_NOCC:
                    nc.gpsimd.collective_compute(
                        "AllGather", mybir.AluOpType.bypass, replica_groups=rg,
                        ins=[x2_own_d.ap().opt()], outs=[x2_tab.ap().opt()])
            if DEBUG_STEPS >= 3:
                spmm_step(3)

    nc.compile()
    return nc


# ---------------------------------------------------------------------------
# entry point
# ---------------------------------------------------------------------------

def kernel(x, edge_row, edge_col, edge_vals, weights, biases):
    global LAST_RESULT
    x = np.asarray(x, dtype=np.float32)
    edge_row = np.asarray(edge_row, dtype=np.int32)
    edge_col = np.asarray(edge_col, dtype=np.int32)
    edge_vals = np.asarray(edge_vals, dtype=np.float32)
    weights = np.asarray(weights, dtype=np.float32)
    biases = np.asarray(biases, dtype=np.float32)

    blocks, NCH, IDXCOLS, nchunk, per_core = _preprocess(
        edge_row.astype(np.int64), edge_col.astype(np.int64), edge_vals)

    # node-major feature table (V_PAD, F), b-major features
    x0 = np.transpose(x, (2, 0, 1)).reshape(V, F)
    x0p = np.zeros((V_PAD, F), dtype=np.float32)
    x0p[:V] = x0
    x0p = x0p.astype(NPDT)

    w_host = np.transpose(weights, (1, 0, 2)).reshape(CIN, K * COUT).astype(NPDT)
    bias_host = biases.reshape(COUT, 1).astype(np.float32)
    iota_host = np.broadcast_to(
        np.arange(128, dtype=np.float32)[None, :], (128, 128)).astype(NPDT).copy()
    ident_host = np.eye(128, dtype=np.float32).astype(NPDT)

    nc = _build_nc(blocks, NCH, IDXCOLS)

    in_maps = []
    for c in range(NCORES):
        idx_np, val_np, roff_np = per_core[c]
        in_maps.append({
            "x0_tab": x0p,
            "x0_own": x0p[c * R:(c + 1) * R].copy(),
            "idxs": idx_np,
            "val1": val_np.astype(NPDT),
            "val2": (2.0 * val_np).astype(NPDT),
            "roff": roff_np.astype(NPDT),
            "iota": iota_host,
            "ident": ident_host,
            "w": w_host,
            "bias": bias_host,
        })

    res = run_bass_kernel_spmd(nc, in_maps, list(range(NCORES)))
    LAST_RESULT = res
    out = np.concatenate([res.results[c]["out"] for c in range(NCORES)], axis=2)
    return np.ascontiguousarray(out[:, :, :V]).astype(np.float32)
